# revision 6
# baseline (speedup 1.0000x reference)
"""Trainium2 Bass kernel for nn_CustomWeightedTensorProduct.

Computes, per edge z (Z=32768), an e3nn-style 'uvu' weighted tensor product:
  out[z,u,k@lo] = (1/sqrt(cnt[lo])) * sum_paths w[z,p,u] * C_p[i,j,k] * x1[z,u,i@l1] * x2[z,j@l2]
with MUL=64, l in {0,1,2} (9 spherical components), 15 paths, per-edge weights.

Strategy (per core, data-parallel over z across 8 cores):
  - edges on partitions, z_in=4 edge-slots per partition per super-tile (512 edges)
  - DVE builds A[(p,i),u] = w_p * x1_i products (broadcast APs), then accumulates
    Y_j[(lo,k),u] = sum c * A via scalar_tensor_tensor with immediate CG coeffs
    (batched into runs where (p,i)/(k) advance arithmetically)
  - PE applies the per-edge x2_j factors as diagonal-stationary matmuls,
    accumulating over j in PSUM (zero-pass first, then start=False accumulation)
  - ACT drains PSUM->SBUF; HWDGE DMA moves I/O
"""

import math
import os
import sys
from contextlib import ExitStack

import numpy as np

for _p in ("/opt/trn_rl_repo", "/root/.axon_site/_ro/trn_rl_repo"):
    if os.path.isdir(_p) and _p not in sys.path:
        sys.path.insert(0, _p)

MUL = 64
Z = 32768
N_CORES = 8
ZC = Z // N_CORES          # 4096 edges per core
LS = (0, 1, 2)
INSTR = tuple(sorted((l1, l2, lo) for l1 in LS for l2 in LS for lo in LS
                     if abs(l1 - l2) <= lo <= l1 + l2))
OFF1 = {0: 0, 1: MUL, 2: 4 * MUL}
OFF2 = {0: 0, 1: 1, 2: 4}
CNT = {0: 3, 1: 6, 2: 6}


# ---- real-basis Wigner 3j (identical math to the module's o3.wigner_3j) ----

def _su2_generators(l):
    m = np.arange(-l, l + 1)
    d = 2 * l + 1
    raise_coef = np.sqrt(l * (l + 1) - m[:-1] * (m[:-1] + 1))
    Jp = np.zeros((d, d), complex)
    Jp[np.arange(1, d), np.arange(0, d - 1)] = raise_coef
    Jm = Jp.conj().T
    Jz = np.diag(m).astype(complex)
    return [(Jp + Jm) / 2.0, (Jp - Jm) / 2.0j, Jz]


def _complex_to_real(l):
    d = 2 * l + 1
    U = np.zeros((d, d), complex)
    U[l, l] = 1.0
    s2 = 1.0 / np.sqrt(2.0)
    for m in range(1, l + 1):
        U[l + m, l - m] = s2
        U[l + m, l + m] = (-1) ** m * s2
        U[l - m, l - m] = 1j * s2
        U[l - m, l + m] = -1j * (-1) ** m * s2
    return U


def _real_generators(l):
    U = _complex_to_real(l)
    return [np.real(-1j * (U @ J @ U.conj().T)) for J in _su2_generators(l)]


def _wigner_3j(l1, l2, l3):
    G1, G2, G3 = _real_generators(l1), _real_generators(l2), _real_generators(l3)
    d1, d2, d3 = 2 * l1 + 1, 2 * l2 + 1, 2 * l3 + 1
    I1, I2, I3 = np.eye(d1), np.eye(d2), np.eye(d3)
    rows = []
    for k in range(3):
        rows.append(np.kron(np.kron(G1[k], I2), I3)
                    + np.kron(np.kron(I1, G2[k]), I3)
                    + np.kron(np.kron(I1, I2), G3[k]))
    K = np.concatenate(rows, axis=0)
    _, _, Vh = np.linalg.svd(K)
    C = Vh[-1].reshape(d1, d2, d3)
    C = C / np.linalg.norm(C)
    flat = C.ravel()
    j = int(np.argmax(np.abs(flat)))
    if flat[j] < 0:
        C = -C
    return C


_W3J = {}
for (l1, l2, lo) in INSTR:
    if (l1, l2, lo) not in _W3J:
        _W3J[(l1, l2, lo)] = (_wigner_3j(l1, l2, lo)
                              * math.sqrt(2 * lo + 1)).astype(np.float64)


# ---- compile-time structure: A columns, Y columns, terms, runs ----

def _build_structure():
    # A columns: (p, i) pairs, path-major
    pi_index = {}
    for p, (l1, l2, lo) in enumerate(INSTR):
        for i in range(2 * l1 + 1):
            pi_index[(p, i)] = len(pi_index)
    n_pi = len(pi_index)  # 51

    # terms: (pi, jg, lo, k, c)
    terms = []
    for p, (l1, l2, lo) in enumerate(INSTR):
        C = _W3J[(l1, l2, lo)] / math.sqrt(CNT[lo])
        for i in range(2 * l1 + 1):
            for j in range(2 * l2 + 1):
                for k in range(2 * lo + 1):
                    c = C[i, j, k]
                    if abs(c) > 1e-12:
                        terms.append((pi_index[(p, i)], OFF2[l2] + j, lo, k,
                                      float(c)))

    # run extraction: bucket by (jg, lo, c); greedy arithmetic chains over
    # (pi, k).  (dpi, 0) is excluded: it would hit the same Y column several
    # times within one instruction (overlapping writes).
    buckets = {}
    for (pi, jg, lo, k, c) in terms:
        buckets.setdefault((jg, lo, round(c, 10)), []).append((pi, k, c))
    runs = []  # (jg, lo, c, pi0, k0, dpi, dk, length)
    for (jg, lo, _cr), items in sorted(buckets.items()):
        c = items[0][2]
        left = sorted((pi, k) for (pi, k, _) in items)
        used = set()
        for deltas in ((1, 1), (1, -1), (0, 1)):
            dpi, dk = deltas
            for (pi, k) in left:
                if (pi, k) in used:
                    continue
                # only start a chain at an element with no predecessor
                if (pi - dpi, k - dk) in set(left) - used:
                    continue
                chain = [(pi, k)]
                while True:
                    nxt = (chain[-1][0] + dpi, chain[-1][1] + dk)
                    if nxt in set(left) - used and nxt not in chain:
                        chain.append(nxt)
                    else:
                        break
                if len(chain) >= 2:
                    used.update(chain)
                    runs.append((jg, lo, c, chain[0][0], chain[0][1],
                                 dpi, dk, len(chain)))
        for (pi, k) in left:
            if (pi, k) not in used:
                runs.append((jg, lo, c, pi, k, 0, 0, 1))

    # verify runs reproduce terms exactly
    chk = []
    for (jg, lo, c, pi0, k0, dpi, dk, L) in runs:
        for t in range(L):
            chk.append((pi0 + t * dpi, jg, lo, k0 + t * dk, c))
    assert sorted((a, b, d, e) for (a, b, d, e, _) in chk) == \
           sorted((a, b, d, e) for (a, b, d, e, _) in terms)

    return pi_index, terms, n_pi, runs


PI_INDEX, TERMS, N_PI, RUNS = _build_structure()


# ---- the Bass kernel builder ----

def build_kernel(zc=ZC, z_in=4):
    import concourse.bass as bass
    import concourse.tile as tile
    from concourse import bacc
    from concourse import mybir
    from concourse.masks import make_identity

    f32 = mybir.dt.float32
    AluOp = mybir.AluOpType
    n_super = zc // (128 * z_in)
    assert zc == n_super * 128 * z_in

    nc = bacc.Bacc("TRN2", target_bir_lowering=False, debug=False)

    def ap_view(t, offset_elems, dims):
        """Manual AP: tile t, extra element offset, free dims [(stride, count)...].
        Partition dim is copied from t."""
        return bass.AP(
            tensor=t.tensor,
            offset=t.offset + offset_elems,
            ap=[list(t.ap[0])] + [[s, n] for (s, n) in dims],
        )

    with tile.TileContext(nc) as tc, ExitStack() as ctx:
        dram = ctx.enter_context(tc.tile_pool(name="dram", bufs=1, space="DRAM"))
        x1_d = dram.tile([zc, MUL * 9], f32, kind="ExternalInput", name="x1",
                         uniquify=False)
        x2_d = dram.tile([zc, 9], f32, kind="ExternalInput", name="x2",
                         uniquify=False)
        w_d = dram.tile([zc, MUL * 15], f32, kind="ExternalInput", name="w",
                        uniquify=False)
        out_d = dram.tile([zc, MUL * 9], f32, kind="ExternalOutput", name="out",
                          uniquify=False)

        consts = ctx.enter_context(tc.tile_pool(name="consts", bufs=1))
        ident = consts.tile([128, 128], f32)
        make_identity(nc, ident)
        zeros = consts.tile([128, 128], f32)
        nc.gpsimd.memset(zeros, 0.0)

        io_pool = ctx.enter_context(tc.tile_pool(name="io", bufs=1))
        a_pool = ctx.enter_context(tc.tile_pool(name="apool", bufs=1))
        y_pool = ctx.enter_context(tc.tile_pool(name="ypool", bufs=1))
        o_pool = ctx.enter_context(tc.tile_pool(name="opool", bufs=1))
        diag_pool = ctx.enter_context(tc.tile_pool(name="diagpool", bufs=3))
        ps_pool = ctx.enter_context(tc.tile_pool(name="pspool", bufs=1,
                                                 space="PSUM"))

        x1_v = x1_d.rearrange("(t p s) f -> t p s f", p=128, s=z_in)
        x2_v = x2_d.rearrange("(t p s) f -> t p s f", p=128, s=z_in)
        w_v = w_d.rearrange("(t p s) f -> t p s f", p=128, s=z_in)
        out_v = out_d.rearrange("(t p s) f -> t p s f", p=128, s=z_in)

        for t in range(n_super):
            X1 = io_pool.tile([128, z_in, 576], f32, tag="x1")
            W = io_pool.tile([128, z_in, 960], f32, tag="w")
            X2 = io_pool.tile([128, z_in, 9], f32, tag="x2")
            nc.sync.dma_start(out=X1, in_=x1_v[t])
            nc.sync.dma_start(out=W, in_=w_v[t])
            nc.sync.dma_start(out=X2, in_=x2_v[t])

            # ---- A build: A[z, s, (p,i), u] = w[z,s,p,u] * x1[z,s,u,i] ----
            A = a_pool.tile([128, z_in, N_PI, 64], f32, tag="A")
            for p, (l1, l2, lo) in enumerate(INSTR):
                d1 = 2 * l1 + 1
                pi0 = PI_INDEX[(p, 0)]
                out_ap = ap_view(A, pi0 * 64,
                                 [(N_PI * 64, z_in), (64, d1), (1, 64)])
                in0 = ap_view(W, p * 64,
                              [(960, z_in), (0, d1), (1, 64)])
                in1 = ap_view(X1, OFF1[l1],
                              [(576, z_in), (1, d1), (d1, 64)])
                nc.vector.tensor_tensor(out=out_ap, in0=in0, in1=in1,
                                        op=AluOp.mult)

            # ---- Y build: Y[z, s, jg, 576@out-layout] += c * A[z, s, pi, u] ----
            # Y[:, s, jg, :] holds sum_{p,i} c * A in the exact output block
            # layout (u-major within each lo block), so PE consumes it as flat
            # contiguous blocks.
            # TensorScalarPtr APs are limited to partition + 2 free dims by the
            # BIR verifier, so each term is one (z_in, 64)-shaped instruction.
            Y = y_pool.tile([128, z_in, 9, 576], f32, tag="Y")
            OFFO = {0: 0, 1: 64, 2: 256}
            written = set()
            for (pi, jg, lo, k, c) in TERMS:
                a_ap = ap_view(A, pi * 64, [(N_PI * 64, z_in), (1, 64)])
                y_ap = ap_view(Y, jg * 576 + OFFO[lo] + k,
                               [(9 * 576, z_in), (2 * lo + 1, 64)])
                if (jg, lo, k) in written:
                    nc.vector.scalar_tensor_tensor(
                        out=y_ap, in0=a_ap, scalar=c, in1=y_ap,
                        op0=AluOp.mult, op1=AluOp.add)
                else:
                    nc.vector.tensor_scalar(
                        out=y_ap, in0=a_ap, scalar1=c, scalar2=None,
                        op0=AluOp.mult)
                    written.add((jg, lo, k))

            # unreferenced (jg, lo, k) columns would feed garbage into PSUM
            assert len(written) == 9 * 9, (len(written))

            # ---- PE: out_psum[z, s, (u,k)@lo] += diag(x2_j) @ Y_j ----
            # PSUM layout: Ps01 [128, z_in, 256] (lo0|lo1 blocks, 1KB/slot),
            #              Ps2  [128, z_in, 512] (lo2 block padded to a bank)
            Ps01 = ps_pool.tile([128, z_in, 256], f32, tag="ps01")
            Ps2 = ps_pool.tile([128, z_in, 512], f32, tag="ps2")
            OFF01 = {0: 0, 1: 64}

            # zero pass (start=True): zeros.T @ anything
            for b in range(z_in * 256 // 512):
                nc.tensor.matmul(out=ap_view(Ps01, b * 512, [(1, 512)]),
                                 lhsT=zeros, rhs=ap_view(X1, 0, [(1, 512)]),
                                 start=True, stop=False, skip_group_check=True)
            for s in range(z_in):
                nc.tensor.matmul(out=ap_view(Ps2, s * 512, [(1, 512)]),
                                 lhsT=zeros, rhs=ap_view(X1, 0, [(1, 512)]),
                                 start=True, stop=False, skip_group_check=True)

            X2f = X2.rearrange("p s f -> p (s f)")
            n_pe = z_in * 9 * 3
            i_pe = 0
            for s in range(z_in):
                for jg in range(9):
                    dg = diag_pool.tile([128, 128], f32, tag="diag")
                    nc.scalar.activation(out=dg, in_=ident,
                                         func=mybir.ActivationFunctionType.Copy,
                                         scale=X2f[:, s * 9 + jg:s * 9 + jg + 1])
                    for lo in LS:
                        sz = 64 * (2 * lo + 1)
                        rhs = ap_view(Y, (s * 9 + jg) * 576 + OFFO[lo],
                                      [(1, sz)])
                        if lo < 2:
                            o_ap = ap_view(Ps01, s * 256 + OFF01[lo], [(1, sz)])
                        else:
                            o_ap = ap_view(Ps2, s * 512, [(1, sz)])
                        i_pe += 1
                        nc.tensor.matmul(out=o_ap, lhsT=dg, rhs=rhs,
                                         start=False, stop=(i_pe == n_pe),
                                         skip_group_check=True)

            # ---- drain PSUM -> SBUF, store ----
            O = o_pool.tile([128, z_in, 576], f32, tag="O")
            nc.scalar.copy(out=ap_view(O, 0, [(576, z_in), (1, 256)]),
                           in_=Ps01)
            nc.scalar.copy(out=ap_view(O, 256, [(576, z_in), (1, 320)]),
                           in_=ap_view(Ps2, 0, [(512, z_in), (1, 320)]))
            nc.sync.dma_start(out=out_v[t], in_=O)

    nc.finalize()
    return nc


# ---- host entry point ----

def kernel(x1: np.ndarray, x2: np.ndarray, w: np.ndarray) -> np.ndarray:
    from concourse.bass_utils import run_bass_kernel_spmd

    x1 = np.ascontiguousarray(x1, dtype=np.float32)
    x2 = np.ascontiguousarray(x2, dtype=np.float32)
    w = np.ascontiguousarray(w, dtype=np.float32)
    assert x1.shape == (Z, 576) and x2.shape == (Z, 9) and w.shape == (Z, 960)

    nc = build_kernel()
    in_maps = []
    for c in range(N_CORES):
        sl = slice(c * ZC, (c + 1) * ZC)
        in_maps.append({"x1": x1[sl], "x2": x2[sl], "w": w[sl]})
    res = run_bass_kernel_spmd(nc, in_maps, core_ids=list(range(N_CORES)))
    return np.concatenate([res.results[c]["out"] for c in range(N_CORES)],
                          axis=0)


# revision 14
# speedup vs baseline: 76.7835x; 76.7835x over previous
"""Trainium2 Bass kernel for nn_CustomWeightedTensorProduct.

Computes, per edge z (Z=32768), an e3nn-style 'uvu' weighted tensor product:
  out[z,u,k@lo] = (1/sqrt(cnt[lo])) * sum_paths w[z,p,u] * C_p[i,j,k] * x1[z,u,i@l1] * x2[z,j@l2]
with MUL=64, l in {0,1,2} (9 spherical components), 15 paths, per-edge weights.

Strategy (per core, data-parallel over z across 8 cores):
  - edges on partitions, z_in=4 edge-slots per partition per super-tile (512 edges)
  - DVE builds A[(p,i),u] = w_p * x1_i products (broadcast APs), then accumulates
    Y_j[(lo,k),u] = sum c * A via scalar_tensor_tensor with immediate CG coeffs
    (batched into runs where (p,i)/(k) advance arithmetically)
  - PE applies the per-edge x2_j factors as diagonal-stationary matmuls,
    accumulating over j in PSUM (zero-pass first, then start=False accumulation)
  - ACT drains PSUM->SBUF; HWDGE DMA moves I/O
"""

import math
import os
import sys
from contextlib import ExitStack

import numpy as np

for _p in ("/opt/trn_rl_repo", "/root/.axon_site/_ro/trn_rl_repo"):
    if os.path.isdir(_p) and _p not in sys.path:
        sys.path.insert(0, _p)

MUL = 64
Z = 32768
N_CORES = 8
ZC = Z // N_CORES          # 4096 edges per core
LS = (0, 1, 2)
INSTR = tuple(sorted((l1, l2, lo) for l1 in LS for l2 in LS for lo in LS
                     if abs(l1 - l2) <= lo <= l1 + l2))
OFF1 = {0: 0, 1: MUL, 2: 4 * MUL}
OFF2 = {0: 0, 1: 1, 2: 4}
CNT = {0: 3, 1: 6, 2: 6}


# ---- real-basis Wigner 3j (identical math to the module's o3.wigner_3j) ----

def _su2_generators(l):
    m = np.arange(-l, l + 1)
    d = 2 * l + 1
    raise_coef = np.sqrt(l * (l + 1) - m[:-1] * (m[:-1] + 1))
    Jp = np.zeros((d, d), complex)
    Jp[np.arange(1, d), np.arange(0, d - 1)] = raise_coef
    Jm = Jp.conj().T
    Jz = np.diag(m).astype(complex)
    return [(Jp + Jm) / 2.0, (Jp - Jm) / 2.0j, Jz]


def _complex_to_real(l):
    d = 2 * l + 1
    U = np.zeros((d, d), complex)
    U[l, l] = 1.0
    s2 = 1.0 / np.sqrt(2.0)
    for m in range(1, l + 1):
        U[l + m, l - m] = s2
        U[l + m, l + m] = (-1) ** m * s2
        U[l - m, l - m] = 1j * s2
        U[l - m, l + m] = -1j * (-1) ** m * s2
    return U


def _real_generators(l):
    U = _complex_to_real(l)
    return [np.real(-1j * (U @ J @ U.conj().T)) for J in _su2_generators(l)]


def _wigner_3j(l1, l2, l3):
    G1, G2, G3 = _real_generators(l1), _real_generators(l2), _real_generators(l3)
    d1, d2, d3 = 2 * l1 + 1, 2 * l2 + 1, 2 * l3 + 1
    I1, I2, I3 = np.eye(d1), np.eye(d2), np.eye(d3)
    rows = []
    for k in range(3):
        rows.append(np.kron(np.kron(G1[k], I2), I3)
                    + np.kron(np.kron(I1, G2[k]), I3)
                    + np.kron(np.kron(I1, I2), G3[k]))
    K = np.concatenate(rows, axis=0)
    _, _, Vh = np.linalg.svd(K)
    C = Vh[-1].reshape(d1, d2, d3)
    C = C / np.linalg.norm(C)
    flat = C.ravel()
    j = int(np.argmax(np.abs(flat)))
    if flat[j] < 0:
        C = -C
    return C


_W3J = {}
for (l1, l2, lo) in INSTR:
    if (l1, l2, lo) not in _W3J:
        _W3J[(l1, l2, lo)] = (_wigner_3j(l1, l2, lo)
                              * math.sqrt(2 * lo + 1)).astype(np.float64)


# ---- compile-time structure: A columns, Y columns, terms, runs ----

def _build_structure():
    # A columns: (p, i) pairs, path-major
    pi_index = {}
    for p, (l1, l2, lo) in enumerate(INSTR):
        for i in range(2 * l1 + 1):
            pi_index[(p, i)] = len(pi_index)
    n_pi = len(pi_index)  # 51

    # terms: (pi, jg, lo, k, c)
    terms = []
    for p, (l1, l2, lo) in enumerate(INSTR):
        C = _W3J[(l1, l2, lo)] / math.sqrt(CNT[lo])
        for i in range(2 * l1 + 1):
            for j in range(2 * l2 + 1):
                for k in range(2 * lo + 1):
                    c = C[i, j, k]
                    if abs(c) > 1e-12:
                        terms.append((pi_index[(p, i)], OFF2[l2] + j, lo, k,
                                      float(c)))

    # run extraction: bucket by (jg, lo, c); greedy arithmetic chains over
    # (pi, k).  (dpi, 0) is excluded: it would hit the same Y column several
    # times within one instruction (overlapping writes).
    buckets = {}
    for (pi, jg, lo, k, c) in terms:
        buckets.setdefault((jg, lo, round(c, 10)), []).append((pi, k, c))
    runs = []  # (jg, lo, c, pi0, k0, dpi, dk, length)
    for (jg, lo, _cr), items in sorted(buckets.items()):
        c = items[0][2]
        left = sorted((pi, k) for (pi, k, _) in items)
        used = set()
        for deltas in ((1, 1), (1, -1), (0, 1)):
            dpi, dk = deltas
            for (pi, k) in left:
                if (pi, k) in used:
                    continue
                # only start a chain at an element with no predecessor
                if (pi - dpi, k - dk) in set(left) - used:
                    continue
                chain = [(pi, k)]
                while True:
                    nxt = (chain[-1][0] + dpi, chain[-1][1] + dk)
                    if nxt in set(left) - used and nxt not in chain:
                        chain.append(nxt)
                    else:
                        break
                if len(chain) >= 2:
                    used.update(chain)
                    runs.append((jg, lo, c, chain[0][0], chain[0][1],
                                 dpi, dk, len(chain)))
        for (pi, k) in left:
            if (pi, k) not in used:
                runs.append((jg, lo, c, pi, k, 0, 0, 1))

    # verify runs reproduce terms exactly
    chk = []
    for (jg, lo, c, pi0, k0, dpi, dk, L) in runs:
        for t in range(L):
            chk.append((pi0 + t * dpi, jg, lo, k0 + t * dk, c))
    assert sorted((a, b, d, e) for (a, b, d, e, _) in chk) == \
           sorted((a, b, d, e) for (a, b, d, e, _) in terms)

    return pi_index, terms, n_pi, runs


PI_INDEX, TERMS, N_PI, RUNS = _build_structure()


# ---- the Bass kernel builder ----

def build_kernel(zc=ZC, z_in=4, loop_n=1, a_engine="vector", y_gp_frac=0.0,
                 diag_engine="scalar"):
    import concourse.bass as bass
    import concourse.tile as tile
    from concourse import bacc
    from concourse import mybir
    from concourse.masks import make_identity

    f32 = mybir.dt.float32
    AluOp = mybir.AluOpType
    n_super = zc // (128 * z_in)
    assert zc == n_super * 128 * z_in

    nc = bacc.Bacc("TRN2", target_bir_lowering=False, debug=False)

    def ap_view(t, offset_elems, dims):
        """Manual AP: tile t, extra element offset, free dims [(stride, count)...].
        Partition dim is copied from t."""
        return bass.AP(
            tensor=t.tensor,
            offset=t.offset + offset_elems,
            ap=[list(t.ap[0])] + [[s, n] for (s, n) in dims],
        )

    with tile.TileContext(nc) as tc, ExitStack() as ctx:
        dram = ctx.enter_context(tc.tile_pool(name="dram", bufs=1, space="DRAM"))
        x1_d = dram.tile([zc, MUL * 9], f32, kind="ExternalInput", name="x1",
                         uniquify=False)
        x2_d = dram.tile([zc, 9], f32, kind="ExternalInput", name="x2",
                         uniquify=False)
        w_d = dram.tile([zc, MUL * 15], f32, kind="ExternalInput", name="w",
                        uniquify=False)
        out_d = dram.tile([zc, MUL * 9], f32, kind="ExternalOutput", name="out",
                          uniquify=False)

        consts = ctx.enter_context(tc.tile_pool(name="consts", bufs=1))
        ident = consts.tile([128, 128], f32)
        make_identity(nc, ident)
        zeros = consts.tile([128, 128], f32)
        nc.gpsimd.memset(zeros, 0.0)

        io_pool = ctx.enter_context(tc.tile_pool(name="io", bufs=1))
        a_pool = ctx.enter_context(tc.tile_pool(name="apool", bufs=1))
        y_pool = ctx.enter_context(tc.tile_pool(name="ypool", bufs=1))
        o_pool = ctx.enter_context(tc.tile_pool(name="opool", bufs=1))
        diag_pool = ctx.enter_context(tc.tile_pool(name="diagpool", bufs=3))
        ps_pool = ctx.enter_context(tc.tile_pool(name="pspool", bufs=1,
                                                 space="PSUM"))

        x1_v = x1_d.rearrange("(t p s) f -> t p s f", p=128, s=z_in)
        x2_v = x2_d.rearrange("(t p s) f -> t p s f", p=128, s=z_in)
        w_v = w_d.rearrange("(t p s) f -> t p s f", p=128, s=z_in)
        out_v = out_d.rearrange("(t p s) f -> t p s f", p=128, s=z_in)

        a_eng = getattr(nc, a_engine)

        # Y-column ownership: move whole accumulation chains (all terms of one
        # (jg,lo,k) column) to GPSIMD until ~y_gp_frac of Y elements moved.
        col_terms = {}
        for (pi, jg, lo, k, c) in TERMS:
            col_terms.setdefault((jg, lo, k), []).append((pi, c))
        total_terms = len(TERMS)
        gp_cols = set()
        moved = 0
        if y_gp_frac > 0:
            for col, ts_ in sorted(col_terms.items(),
                                   key=lambda kv: -len(kv[1])):
                if moved / total_terms >= y_gp_frac:
                    break
                gp_cols.add(col)
                moved += len(ts_)

        loop_ctx = tc.For_i(0, loop_n, 1) if loop_n > 1 else None
        if loop_ctx is not None:
            loop_ctx.__enter__()
        for t in range(n_super):
            X1 = io_pool.tile([128, z_in, 576], f32, tag="x1")
            W = io_pool.tile([128, z_in, 960], f32, tag="w")
            X2 = io_pool.tile([128, z_in, 9], f32, tag="x2")
            nc.sync.dma_start(out=X1, in_=x1_v[t])
            nc.sync.dma_start(out=W, in_=w_v[t])
            nc.sync.dma_start(out=X2, in_=x2_v[t])

            # ---- A build: A[z, s, (p,i), u] = w[z,s,p,u] * x1[z,s,u,i] ----
            A = a_pool.tile([128, z_in, N_PI, 64], f32, tag="A")
            for p, (l1, l2, lo) in enumerate(INSTR):
                d1 = 2 * l1 + 1
                pi0 = PI_INDEX[(p, 0)]
                out_ap = ap_view(A, pi0 * 64,
                                 [(N_PI * 64, z_in), (64, d1), (1, 64)])
                in0 = ap_view(W, p * 64,
                              [(960, z_in), (0, d1), (1, 64)])
                in1 = ap_view(X1, OFF1[l1],
                              [(576, z_in), (1, d1), (d1, 64)])
                a_eng.tensor_tensor(out=out_ap, in0=in0, in1=in1,
                                    op=AluOp.mult)

            # ---- Y build: Y[z, s, jg, 576@out-layout] += c * A[z, s, pi, u] ----
            # Y[:, s, jg, :] holds sum_{p,i} c * A in the exact output block
            # layout (u-major within each lo block), so PE consumes it as flat
            # contiguous blocks.
            # TensorScalarPtr APs are limited to partition + 2 free dims by the
            # BIR verifier, so each term is one (z_in, 64)-shaped instruction.
            Y = y_pool.tile([128, z_in, 9, 576], f32, tag="Y")
            OFFO = {0: 0, 1: 64, 2: 256}
            written = set()
            for (pi, jg, lo, k, c) in TERMS:
                a_ap = ap_view(A, pi * 64, [(N_PI * 64, z_in), (1, 64)])
                y_ap = ap_view(Y, jg * 576 + OFFO[lo] + k,
                               [(9 * 576, z_in), (2 * lo + 1, 64)])
                eng = nc.gpsimd if (jg, lo, k) in gp_cols else nc.vector
                if (jg, lo, k) in written:
                    eng.scalar_tensor_tensor(
                        out=y_ap, in0=a_ap, scalar=c, in1=y_ap,
                        op0=AluOp.mult, op1=AluOp.add)
                else:
                    eng.tensor_scalar(
                        out=y_ap, in0=a_ap, scalar1=c, scalar2=None,
                        op0=AluOp.mult)
                    written.add((jg, lo, k))

            # unreferenced (jg, lo, k) columns would feed garbage into PSUM
            assert len(written) == 9 * 9, (len(written))

            # ---- PE: out_psum[z, s, (u,k)@lo] += diag(x2_j) @ Y_j ----
            # PSUM layout: Ps01 [128, z_in, 256] (lo0|lo1 blocks, 1KB/slot),
            #              Ps2  [128, z_in, 512] (lo2 block padded to a bank)
            Ps01 = ps_pool.tile([128, z_in, 256], f32, tag="ps01")
            Ps2 = ps_pool.tile([128, z_in, 512], f32, tag="ps2")
            OFF01 = {0: 0, 1: 64}

            # zero pass (start=True): zeros.T @ anything
            for b in range(z_in * 256 // 512):
                nc.tensor.matmul(out=ap_view(Ps01, b * 512, [(1, 512)]),
                                 lhsT=zeros, rhs=ap_view(X1, 0, [(1, 512)]),
                                 start=True, stop=False, skip_group_check=True)
            for s in range(z_in):
                nc.tensor.matmul(out=ap_view(Ps2, s * 512, [(1, 512)]),
                                 lhsT=zeros, rhs=ap_view(X1, 0, [(1, 512)]),
                                 start=True, stop=False, skip_group_check=True)

            X2f = X2.rearrange("p s f -> p (s f)")
            n_pe = z_in * 9 * 3
            i_pe = 0
            for s in range(z_in):
                for jg in range(9):
                    dg = diag_pool.tile([128, 128], f32, tag="diag")
                    if diag_engine == "scalar":
                        nc.scalar.activation(
                            out=dg, in_=ident,
                            func=mybir.ActivationFunctionType.Copy,
                            scale=X2f[:, s * 9 + jg:s * 9 + jg + 1])
                    else:
                        getattr(nc, diag_engine).tensor_scalar(
                            out=dg, in0=ident,
                            scalar1=X2f[:, s * 9 + jg:s * 9 + jg + 1],
                            scalar2=None, op0=AluOp.mult)
                    for lo in LS:
                        sz = 64 * (2 * lo + 1)
                        rhs = ap_view(Y, (s * 9 + jg) * 576 + OFFO[lo],
                                      [(1, sz)])
                        if lo < 2:
                            o_ap = ap_view(Ps01, s * 256 + OFF01[lo], [(1, sz)])
                        else:
                            o_ap = ap_view(Ps2, s * 512, [(1, sz)])
                        i_pe += 1
                        nc.tensor.matmul(out=o_ap, lhsT=dg, rhs=rhs,
                                         start=False, stop=(i_pe == n_pe),
                                         skip_group_check=True)

            # ---- drain PSUM -> SBUF, store ----
            O = o_pool.tile([128, z_in, 576], f32, tag="O")
            nc.scalar.copy(out=ap_view(O, 0, [(576, z_in), (1, 256)]),
                           in_=Ps01)
            nc.scalar.copy(out=ap_view(O, 256, [(576, z_in), (1, 320)]),
                           in_=ap_view(Ps2, 0, [(512, z_in), (1, 320)]))
            nc.sync.dma_start(out=out_v[t], in_=O)

        if loop_ctx is not None:
            loop_ctx.__exit__(None, None, None)

    nc.finalize()
    return nc


# ---- host entry point ----

# A-products on GPSIMD (TensorTensor ucode is legal on Pool; TensorScalarPtr
# is not, so Y stays fully on DVE), diag builds + drains on ACT
BEST_CFG = dict(a_engine="gpsimd", y_gp_frac=0.0, diag_engine="scalar")


def kernel(x1: np.ndarray, x2: np.ndarray, w: np.ndarray) -> np.ndarray:
    from concourse.bass_utils import run_bass_kernel_spmd

    x1 = np.ascontiguousarray(x1, dtype=np.float32)
    x2 = np.ascontiguousarray(x2, dtype=np.float32)
    w = np.ascontiguousarray(w, dtype=np.float32)
    assert x1.shape == (Z, 576) and x2.shape == (Z, 9) and w.shape == (Z, 960)

    nc = build_kernel(**BEST_CFG)
    in_maps = []
    for c in range(N_CORES):
        sl = slice(c * ZC, (c + 1) * ZC)
        in_maps.append({"x1": x1[sl], "x2": x2[sl], "w": w[sl]})
    res = run_bass_kernel_spmd(nc, in_maps, core_ids=list(range(N_CORES)))
    return np.concatenate([res.results[c]["out"] for c in range(N_CORES)],
                          axis=0)


# revision 21
# speedup vs baseline: 84.5260x; 1.1008x over previous
"""Trainium2 Bass kernel for nn_CustomWeightedTensorProduct.

Computes, per edge z (Z=32768), an e3nn-style 'uvu' weighted tensor product:
  out[z,u,k@lo] = (1/sqrt(cnt[lo])) * sum_paths w[z,p,u] * C_p[i,j,k] * x1[z,u,i@l1] * x2[z,j@l2]
with MUL=64, l in {0,1,2} (9 spherical components), 15 paths, per-edge weights.

Strategy (per core, data-parallel over z across 8 cores):
  - edges on partitions, z_in=4 edge-slots per partition per super-tile (512 edges)
  - DVE builds A[(p,i),u] = w_p * x1_i products (broadcast APs), then accumulates
    Y_j[(lo,k),u] = sum c * A via scalar_tensor_tensor with immediate CG coeffs
    (batched into runs where (p,i)/(k) advance arithmetically)
  - PE applies the per-edge x2_j factors as diagonal-stationary matmuls,
    accumulating over j in PSUM (zero-pass first, then start=False accumulation)
  - ACT drains PSUM->SBUF; HWDGE DMA moves I/O
"""

import math
import os
import sys
from contextlib import ExitStack

import numpy as np

for _p in ("/opt/trn_rl_repo", "/root/.axon_site/_ro/trn_rl_repo"):
    if os.path.isdir(_p) and _p not in sys.path:
        sys.path.insert(0, _p)

MUL = 64
Z = 32768
N_CORES = 8
ZC = Z // N_CORES          # 4096 edges per core
LS = (0, 1, 2)
INSTR = tuple(sorted((l1, l2, lo) for l1 in LS for l2 in LS for lo in LS
                     if abs(l1 - l2) <= lo <= l1 + l2))
OFF1 = {0: 0, 1: MUL, 2: 4 * MUL}
OFF2 = {0: 0, 1: 1, 2: 4}
CNT = {0: 3, 1: 6, 2: 6}


# ---- real-basis Wigner 3j (identical math to the module's o3.wigner_3j) ----

def _su2_generators(l):
    m = np.arange(-l, l + 1)
    d = 2 * l + 1
    raise_coef = np.sqrt(l * (l + 1) - m[:-1] * (m[:-1] + 1))
    Jp = np.zeros((d, d), complex)
    Jp[np.arange(1, d), np.arange(0, d - 1)] = raise_coef
    Jm = Jp.conj().T
    Jz = np.diag(m).astype(complex)
    return [(Jp + Jm) / 2.0, (Jp - Jm) / 2.0j, Jz]


def _complex_to_real(l):
    d = 2 * l + 1
    U = np.zeros((d, d), complex)
    U[l, l] = 1.0
    s2 = 1.0 / np.sqrt(2.0)
    for m in range(1, l + 1):
        U[l + m, l - m] = s2
        U[l + m, l + m] = (-1) ** m * s2
        U[l - m, l - m] = 1j * s2
        U[l - m, l + m] = -1j * (-1) ** m * s2
    return U


def _real_generators(l):
    U = _complex_to_real(l)
    return [np.real(-1j * (U @ J @ U.conj().T)) for J in _su2_generators(l)]


def _wigner_3j(l1, l2, l3):
    G1, G2, G3 = _real_generators(l1), _real_generators(l2), _real_generators(l3)
    d1, d2, d3 = 2 * l1 + 1, 2 * l2 + 1, 2 * l3 + 1
    I1, I2, I3 = np.eye(d1), np.eye(d2), np.eye(d3)
    rows = []
    for k in range(3):
        rows.append(np.kron(np.kron(G1[k], I2), I3)
                    + np.kron(np.kron(I1, G2[k]), I3)
                    + np.kron(np.kron(I1, I2), G3[k]))
    K = np.concatenate(rows, axis=0)
    _, _, Vh = np.linalg.svd(K)
    C = Vh[-1].reshape(d1, d2, d3)
    C = C / np.linalg.norm(C)
    flat = C.ravel()
    j = int(np.argmax(np.abs(flat)))
    if flat[j] < 0:
        C = -C
    return C


_W3J = {}
for (l1, l2, lo) in INSTR:
    if (l1, l2, lo) not in _W3J:
        _W3J[(l1, l2, lo)] = (_wigner_3j(l1, l2, lo)
                              * math.sqrt(2 * lo + 1)).astype(np.float64)


# ---- compile-time structure: A columns, Y columns, terms, runs ----

def _build_structure():
    # A columns: (p, i) pairs, path-major
    pi_index = {}
    for p, (l1, l2, lo) in enumerate(INSTR):
        for i in range(2 * l1 + 1):
            pi_index[(p, i)] = len(pi_index)
    n_pi = len(pi_index)  # 51

    # terms: (pi, jg, lo, k, c)
    terms = []
    for p, (l1, l2, lo) in enumerate(INSTR):
        C = _W3J[(l1, l2, lo)] / math.sqrt(CNT[lo])
        for i in range(2 * l1 + 1):
            for j in range(2 * l2 + 1):
                for k in range(2 * lo + 1):
                    c = C[i, j, k]
                    if abs(c) > 1e-12:
                        terms.append((pi_index[(p, i)], OFF2[l2] + j, lo, k,
                                      float(c)))

    # run extraction: bucket by (jg, lo, c); greedy arithmetic chains over
    # (pi, k).  (dpi, 0) is excluded: it would hit the same Y column several
    # times within one instruction (overlapping writes).
    buckets = {}
    for (pi, jg, lo, k, c) in terms:
        buckets.setdefault((jg, lo, round(c, 10)), []).append((pi, k, c))
    runs = []  # (jg, lo, c, pi0, k0, dpi, dk, length)
    for (jg, lo, _cr), items in sorted(buckets.items()):
        c = items[0][2]
        left = sorted((pi, k) for (pi, k, _) in items)
        used = set()
        for deltas in ((1, 1), (1, -1), (0, 1)):
            dpi, dk = deltas
            for (pi, k) in left:
                if (pi, k) in used:
                    continue
                # only start a chain at an element with no predecessor
                if (pi - dpi, k - dk) in set(left) - used:
                    continue
                chain = [(pi, k)]
                while True:
                    nxt = (chain[-1][0] + dpi, chain[-1][1] + dk)
                    if nxt in set(left) - used and nxt not in chain:
                        chain.append(nxt)
                    else:
                        break
                if len(chain) >= 2:
                    used.update(chain)
                    runs.append((jg, lo, c, chain[0][0], chain[0][1],
                                 dpi, dk, len(chain)))
        for (pi, k) in left:
            if (pi, k) not in used:
                runs.append((jg, lo, c, pi, k, 0, 0, 1))

    # verify runs reproduce terms exactly
    chk = []
    for (jg, lo, c, pi0, k0, dpi, dk, L) in runs:
        for t in range(L):
            chk.append((pi0 + t * dpi, jg, lo, k0 + t * dk, c))
    assert sorted((a, b, d, e) for (a, b, d, e, _) in chk) == \
           sorted((a, b, d, e) for (a, b, d, e, _) in terms)

    return pi_index, terms, n_pi, runs


PI_INDEX, TERMS, N_PI, RUNS = _build_structure()


# ---- the Bass kernel builder ----

def build_kernel(zc=ZC, z_in=4, loop_n=1, a_engine="vector", y_gp_frac=0.0,
                 diag_engine="scalar", fw_act_n=0, k_major=False):
    import concourse.bass as bass
    import concourse.tile as tile
    from concourse import bacc
    from concourse import mybir
    from concourse.masks import make_identity

    f32 = mybir.dt.float32
    AluOp = mybir.AluOpType
    n_super = zc // (128 * z_in)
    assert zc == n_super * 128 * z_in

    nc = bacc.Bacc("TRN2", target_bir_lowering=False, debug=False)

    def ap_view(t, offset_elems, dims):
        """Manual AP: tile t, extra element offset, free dims [(stride, count)...].
        Partition dim is copied from t."""
        return bass.AP(
            tensor=t.tensor,
            offset=t.offset + offset_elems,
            ap=[list(t.ap[0])] + [[s, n] for (s, n) in dims],
        )

    with tile.TileContext(nc) as tc, ExitStack() as ctx:
        dram = ctx.enter_context(tc.tile_pool(name="dram", bufs=1, space="DRAM"))
        x1_d = dram.tile([zc, MUL * 9], f32, kind="ExternalInput", name="x1",
                         uniquify=False)
        x2_d = dram.tile([zc, 9], f32, kind="ExternalInput", name="x2",
                         uniquify=False)
        w_d = dram.tile([zc, MUL * 15], f32, kind="ExternalInput", name="w",
                        uniquify=False)
        out_d = dram.tile([zc, MUL * 9], f32, kind="ExternalOutput", name="out",
                          uniquify=False)

        consts = ctx.enter_context(tc.tile_pool(name="consts", bufs=1))
        ident = consts.tile([128, 128], f32)
        make_identity(nc, ident)
        zeros = consts.tile([128, 128], f32)
        nc.gpsimd.memset(zeros, 0.0)

        io_pool = ctx.enter_context(tc.tile_pool(name="io", bufs=1))
        a_pool = ctx.enter_context(tc.tile_pool(name="apool", bufs=1))
        y_pool = ctx.enter_context(tc.tile_pool(name="ypool", bufs=1))
        o_pool = ctx.enter_context(tc.tile_pool(name="opool", bufs=1))
        diag_pool = ctx.enter_context(tc.tile_pool(name="diagpool", bufs=3))
        ps_pool = ctx.enter_context(tc.tile_pool(name="pspool", bufs=1,
                                                 space="PSUM"))

        x1_v = x1_d.rearrange("(t p s) f -> t p s f", p=128, s=z_in)
        x2_v = x2_d.rearrange("(t p s) f -> t p s f", p=128, s=z_in)
        w_v = w_d.rearrange("(t p s) f -> t p s f", p=128, s=z_in)
        out_v = out_d.rearrange("(t p s) f -> t p s f", p=128, s=z_in)

        a_eng = getattr(nc, a_engine)

        # Y-column ownership: move whole accumulation chains (all terms of one
        # (jg,lo,k) column) to GPSIMD until ~y_gp_frac of Y elements moved.
        col_terms = {}
        for (pi, jg, lo, k, c) in TERMS:
            col_terms.setdefault((jg, lo, k), []).append((pi, c))
        total_terms = len(TERMS)
        gp_cols = set()
        moved = 0
        if y_gp_frac > 0:
            for col, ts_ in sorted(col_terms.items(),
                                   key=lambda kv: -len(kv[1])):
                if moved / total_terms >= y_gp_frac:
                    break
                gp_cols.add(col)
                moved += len(ts_)

        loop_ctx = tc.For_i(0, loop_n, 1) if loop_n > 1 else None
        if loop_ctx is not None:
            loop_ctx.__enter__()
        for t in range(n_super):
            X1 = io_pool.tile([128, z_in, 576], f32, tag="x1")
            W = io_pool.tile([128, z_in, 960], f32, tag="w")
            X2 = io_pool.tile([128, z_in, 9], f32, tag="x2")
            nc.sync.dma_start(out=X1, in_=x1_v[t])
            nc.sync.dma_start(out=W, in_=w_v[t])
            nc.sync.dma_start(out=X2, in_=x2_v[t])

            # ---- A build: A[z, s, (p,i), u] = w[z,s,p,u] * x1[z,s,u,i] ----
            A = a_pool.tile([128, z_in, N_PI, 64], f32, tag="A")
            for p, (l1, l2, lo) in enumerate(INSTR):
                d1 = 2 * l1 + 1
                pi0 = PI_INDEX[(p, 0)]
                out_ap = ap_view(A, pi0 * 64,
                                 [(N_PI * 64, z_in), (64, d1), (1, 64)])
                in0 = ap_view(W, p * 64,
                              [(960, z_in), (0, d1), (1, 64)])
                in1 = ap_view(X1, OFF1[l1],
                              [(576, z_in), (1, d1), (d1, 64)])
                a_eng.tensor_tensor(out=out_ap, in0=in0, in1=in1,
                                    op=AluOp.mult)

            # ---- Y build: Y[z, s, jg, 576@out-layout] += c * A[z, s, pi, u] ----
            # Y[:, s, jg, :] holds sum_{p,i} c * A in the exact output block
            # layout (u-major within each lo block), so PE consumes it as flat
            # contiguous blocks.
            # TensorScalarPtr APs are limited to partition + 2 free dims by the
            # BIR verifier, so each term is one (z_in, 64)-shaped instruction.
            Y = y_pool.tile([128, z_in, 9, 576], f32, tag="Y")
            OFFO = {0: 0, 1: 64, 2: 256}
            written = set()
            n_fw_act = 0
            for (pi, jg, lo, k, c) in TERMS:
                a_ap = ap_view(A, pi * 64, [(N_PI * 64, z_in), (1, 64)])
                if k_major:
                    # contiguous per-(jg,lo,k) column: unit-stride out lets
                    # single-src tensor_scalar first-writes hit the 2x mode
                    y_ap = ap_view(Y, jg * 576 + OFFO[lo] + k * 64,
                                   [(9 * 576, z_in), (1, 64)])
                else:
                    y_ap = ap_view(Y, jg * 576 + OFFO[lo] + k,
                                   [(9 * 576, z_in), (2 * lo + 1, 64)])
                eng = nc.gpsimd if (jg, lo, k) in gp_cols else nc.vector
                if (jg, lo, k) in written:
                    eng.scalar_tensor_tensor(
                        out=y_ap, in0=a_ap, scalar=c, in1=y_ap,
                        op0=AluOp.mult, op1=AluOp.add)
                elif n_fw_act < fw_act_n:
                    # first write of a column as an affine ACT op keeps the
                    # accumulation chains off the (bottleneck) vector engine
                    n_fw_act += 1
                    nc.scalar.activation(
                        out=y_ap, in_=a_ap,
                        func=mybir.ActivationFunctionType.Copy, scale=c)
                    written.add((jg, lo, k))
                else:
                    eng.tensor_scalar(
                        out=y_ap, in0=a_ap, scalar1=c, scalar2=None,
                        op0=AluOp.mult)
                    written.add((jg, lo, k))

            # unreferenced (jg, lo, k) columns would feed garbage into PSUM
            assert len(written) == 9 * 9, (len(written))

            # ---- PE: out_psum[z, s, (u,k)@lo] += diag(x2_j) @ Y_j ----
            # PSUM layout: Ps01 [128, z_in, 256] (lo0|lo1 blocks, 1KB/slot),
            #              Ps2  [128, z_in, 512] (lo2 block padded to a bank)
            Ps01 = ps_pool.tile([128, z_in, 256], f32, tag="ps01")
            Ps2 = ps_pool.tile([128, z_in, 512], f32, tag="ps2")
            OFF01 = {0: 0, 1: 64}

            # zero pass (start=True): zeros.T @ anything
            for b in range(z_in * 256 // 512):
                nc.tensor.matmul(out=ap_view(Ps01, b * 512, [(1, 512)]),
                                 lhsT=zeros, rhs=ap_view(X1, 0, [(1, 512)]),
                                 start=True, stop=False, skip_group_check=True)
            for s in range(z_in):
                nc.tensor.matmul(out=ap_view(Ps2, s * 512, [(1, 512)]),
                                 lhsT=zeros, rhs=ap_view(X1, 0, [(1, 512)]),
                                 start=True, stop=False, skip_group_check=True)

            X2f = X2.rearrange("p s f -> p (s f)")
            n_pe = z_in * 9 * 3
            i_pe = 0
            for s in range(z_in):
                for jg in range(9):
                    dg = diag_pool.tile([128, 128], f32, tag="diag")
                    if diag_engine == "scalar":
                        nc.scalar.activation(
                            out=dg, in_=ident,
                            func=mybir.ActivationFunctionType.Copy,
                            scale=X2f[:, s * 9 + jg:s * 9 + jg + 1])
                    else:
                        getattr(nc, diag_engine).tensor_scalar(
                            out=dg, in0=ident,
                            scalar1=X2f[:, s * 9 + jg:s * 9 + jg + 1],
                            scalar2=None, op0=AluOp.mult)
                    for lo in LS:
                        sz = 64 * (2 * lo + 1)
                        rhs = ap_view(Y, (s * 9 + jg) * 576 + OFFO[lo],
                                      [(1, sz)])
                        if lo < 2:
                            o_ap = ap_view(Ps01, s * 256 + OFF01[lo], [(1, sz)])
                        else:
                            o_ap = ap_view(Ps2, s * 512, [(1, sz)])
                        i_pe += 1
                        # under k_major both Y and PSUM blocks are k-major,
                        # so the flat contiguous views stay correct
                        nc.tensor.matmul(out=o_ap, lhsT=dg, rhs=rhs,
                                         start=False, stop=(i_pe == n_pe),
                                         skip_group_check=True)

            # ---- drain PSUM -> SBUF, store ----
            O = o_pool.tile([128, z_in, 576], f32, tag="O")
            if not k_major:
                nc.scalar.copy(out=ap_view(O, 0, [(576, z_in), (1, 256)]),
                               in_=Ps01)
                nc.scalar.copy(out=ap_view(O, 256, [(576, z_in), (1, 320)]),
                               in_=ap_view(Ps2, 0, [(512, z_in), (1, 320)]))
            else:
                # PSUM blocks are k-major; transpose each back to the u-major
                # output layout during the drain
                for s in range(z_in):
                    nc.scalar.copy(
                        out=ap_view(O, s * 576, [(1, 64)]),
                        in_=ap_view(Ps01, s * 256, [(1, 64)]))
                    for lo in (1, 2):
                        nk = 2 * lo + 1
                        src = (ap_view(Ps01, s * 256 + 64, [(64, nk), (1, 64)])
                               if lo == 1 else
                               ap_view(Ps2, s * 512, [(64, nk), (1, 64)]))
                        nc.scalar.copy(
                            out=ap_view(O, s * 576 + OFFO[lo],
                                        [(1, nk), (nk, 64)]),
                            in_=src)
            nc.sync.dma_start(out=out_v[t], in_=O)

        if loop_ctx is not None:
            loop_ctx.__exit__(None, None, None)

    nc.finalize()
    return nc


# ---- host entry point ----

# A-products on GPSIMD (TensorTensor ucode is legal on Pool; TensorScalarPtr
# is not, so Y stays fully on DVE), diag builds + drains on ACT.  k_major
# stores Y/PSUM blocks k-major so every Y column is contiguous: single-src
# tensor_scalar first-writes hit the DVE 2x port mode (measured ~504us vs
# ~841us per iteration for the u-major layout under identical host load).
BEST_CFG = dict(a_engine="gpsimd", y_gp_frac=0.0, diag_engine="scalar",
                k_major=True)


def kernel(x1: np.ndarray, x2: np.ndarray, w: np.ndarray) -> np.ndarray:
    from concourse.bass_utils import run_bass_kernel_spmd

    x1 = np.ascontiguousarray(x1, dtype=np.float32)
    x2 = np.ascontiguousarray(x2, dtype=np.float32)
    w = np.ascontiguousarray(w, dtype=np.float32)
    assert x1.shape == (Z, 576) and x2.shape == (Z, 9) and w.shape == (Z, 960)

    nc = build_kernel(**BEST_CFG)
    in_maps = []
    for c in range(N_CORES):
        sl = slice(c * ZC, (c + 1) * ZC)
        in_maps.append({"x1": x1[sl], "x2": x2[sl], "w": w[sl]})
    res = run_bass_kernel_spmd(nc, in_maps, core_ids=list(range(N_CORES)))
    return np.concatenate([res.results[c]["out"] for c in range(N_CORES)],
                          axis=0)


# revision 26
# speedup vs baseline: 88.3835x; 1.0456x over previous
"""Trainium2 Bass kernel for nn_CustomWeightedTensorProduct.

Computes, per edge z (Z=32768), an e3nn-style 'uvu' weighted tensor product:
  out[z,u,k@lo] = (1/sqrt(cnt[lo])) * sum_paths w[z,p,u] * C_p[i,j,k] * x1[z,u,i@l1] * x2[z,j@l2]
with MUL=64, l in {0,1,2} (9 spherical components), 15 paths, per-edge weights.

Strategy (per core, data-parallel over z across 8 cores):
  - edges on partitions, z_in=4 edge-slots per partition per super-tile (512 edges)
  - DVE builds A[(p,i),u] = w_p * x1_i products (broadcast APs), then accumulates
    Y_j[(lo,k),u] = sum c * A via scalar_tensor_tensor with immediate CG coeffs
    (batched into runs where (p,i)/(k) advance arithmetically)
  - PE applies the per-edge x2_j factors as diagonal-stationary matmuls,
    accumulating over j in PSUM (zero-pass first, then start=False accumulation)
  - ACT drains PSUM->SBUF; HWDGE DMA moves I/O
"""

import math
import os
import sys
from contextlib import ExitStack

import numpy as np

for _p in ("/opt/trn_rl_repo", "/root/.axon_site/_ro/trn_rl_repo"):
    if os.path.isdir(_p) and _p not in sys.path:
        sys.path.insert(0, _p)

MUL = 64
Z = 32768
N_CORES = 8
ZC = Z // N_CORES          # 4096 edges per core
LS = (0, 1, 2)
INSTR = tuple(sorted((l1, l2, lo) for l1 in LS for l2 in LS for lo in LS
                     if abs(l1 - l2) <= lo <= l1 + l2))
OFF1 = {0: 0, 1: MUL, 2: 4 * MUL}
OFF2 = {0: 0, 1: 1, 2: 4}
CNT = {0: 3, 1: 6, 2: 6}


# ---- real-basis Wigner 3j (identical math to the module's o3.wigner_3j) ----

def _su2_generators(l):
    m = np.arange(-l, l + 1)
    d = 2 * l + 1
    raise_coef = np.sqrt(l * (l + 1) - m[:-1] * (m[:-1] + 1))
    Jp = np.zeros((d, d), complex)
    Jp[np.arange(1, d), np.arange(0, d - 1)] = raise_coef
    Jm = Jp.conj().T
    Jz = np.diag(m).astype(complex)
    return [(Jp + Jm) / 2.0, (Jp - Jm) / 2.0j, Jz]


def _complex_to_real(l):
    d = 2 * l + 1
    U = np.zeros((d, d), complex)
    U[l, l] = 1.0
    s2 = 1.0 / np.sqrt(2.0)
    for m in range(1, l + 1):
        U[l + m, l - m] = s2
        U[l + m, l + m] = (-1) ** m * s2
        U[l - m, l - m] = 1j * s2
        U[l - m, l + m] = -1j * (-1) ** m * s2
    return U


def _real_generators(l):
    U = _complex_to_real(l)
    return [np.real(-1j * (U @ J @ U.conj().T)) for J in _su2_generators(l)]


def _wigner_3j(l1, l2, l3):
    G1, G2, G3 = _real_generators(l1), _real_generators(l2), _real_generators(l3)
    d1, d2, d3 = 2 * l1 + 1, 2 * l2 + 1, 2 * l3 + 1
    I1, I2, I3 = np.eye(d1), np.eye(d2), np.eye(d3)
    rows = []
    for k in range(3):
        rows.append(np.kron(np.kron(G1[k], I2), I3)
                    + np.kron(np.kron(I1, G2[k]), I3)
                    + np.kron(np.kron(I1, I2), G3[k]))
    K = np.concatenate(rows, axis=0)
    _, _, Vh = np.linalg.svd(K)
    C = Vh[-1].reshape(d1, d2, d3)
    C = C / np.linalg.norm(C)
    flat = C.ravel()
    j = int(np.argmax(np.abs(flat)))
    if flat[j] < 0:
        C = -C
    return C


_W3J = {}
for (l1, l2, lo) in INSTR:
    if (l1, l2, lo) not in _W3J:
        _W3J[(l1, l2, lo)] = (_wigner_3j(l1, l2, lo)
                              * math.sqrt(2 * lo + 1)).astype(np.float64)


# ---- compile-time structure: A columns, Y columns, terms, runs ----

def _build_structure():
    # A columns: (p, i) pairs, path-major
    pi_index = {}
    for p, (l1, l2, lo) in enumerate(INSTR):
        for i in range(2 * l1 + 1):
            pi_index[(p, i)] = len(pi_index)
    n_pi = len(pi_index)  # 51

    # terms: (pi, jg, lo, k, c)
    terms = []
    for p, (l1, l2, lo) in enumerate(INSTR):
        C = _W3J[(l1, l2, lo)] / math.sqrt(CNT[lo])
        for i in range(2 * l1 + 1):
            for j in range(2 * l2 + 1):
                for k in range(2 * lo + 1):
                    c = C[i, j, k]
                    if abs(c) > 1e-12:
                        terms.append((pi_index[(p, i)], OFF2[l2] + j, lo, k,
                                      float(c)))

    # run extraction: bucket by (jg, lo, c); greedy arithmetic chains over
    # (pi, k).  (dpi, 0) is excluded: it would hit the same Y column several
    # times within one instruction (overlapping writes).
    buckets = {}
    for (pi, jg, lo, k, c) in terms:
        buckets.setdefault((jg, lo, round(c, 10)), []).append((pi, k, c))
    runs = []  # (jg, lo, c, pi0, k0, dpi, dk, length)
    for (jg, lo, _cr), items in sorted(buckets.items()):
        c = items[0][2]
        left = sorted((pi, k) for (pi, k, _) in items)
        used = set()
        for deltas in ((1, 1), (1, -1), (0, 1)):
            dpi, dk = deltas
            for (pi, k) in left:
                if (pi, k) in used:
                    continue
                # only start a chain at an element with no predecessor
                if (pi - dpi, k - dk) in set(left) - used:
                    continue
                chain = [(pi, k)]
                while True:
                    nxt = (chain[-1][0] + dpi, chain[-1][1] + dk)
                    if nxt in set(left) - used and nxt not in chain:
                        chain.append(nxt)
                    else:
                        break
                if len(chain) >= 2:
                    used.update(chain)
                    runs.append((jg, lo, c, chain[0][0], chain[0][1],
                                 dpi, dk, len(chain)))
        for (pi, k) in left:
            if (pi, k) not in used:
                runs.append((jg, lo, c, pi, k, 0, 0, 1))

    # verify runs reproduce terms exactly
    chk = []
    for (jg, lo, c, pi0, k0, dpi, dk, L) in runs:
        for t in range(L):
            chk.append((pi0 + t * dpi, jg, lo, k0 + t * dk, c))
    assert sorted((a, b, d, e) for (a, b, d, e, _) in chk) == \
           sorted((a, b, d, e) for (a, b, d, e, _) in terms)

    return pi_index, terms, n_pi, runs


PI_INDEX, TERMS, N_PI, RUNS = _build_structure()


# ---- the Bass kernel builder ----

def build_kernel(zc=ZC, z_in=4, loop_n=1, a_engine="vector", y_gp_frac=0.0,
                 diag_engine="scalar", fw_act_n=0, k_major=False):
    import concourse.bass as bass
    import concourse.tile as tile
    from concourse import bacc
    from concourse import mybir
    from concourse.masks import make_identity

    f32 = mybir.dt.float32
    AluOp = mybir.AluOpType
    n_super = zc // (128 * z_in)
    assert zc == n_super * 128 * z_in

    nc = bacc.Bacc("TRN2", target_bir_lowering=False, debug=False)

    def ap_view(t, offset_elems, dims):
        """Manual AP: tile t, extra element offset, free dims [(stride, count)...].
        Partition dim is copied from t."""
        return bass.AP(
            tensor=t.tensor,
            offset=t.offset + offset_elems,
            ap=[list(t.ap[0])] + [[s, n] for (s, n) in dims],
        )

    with tile.TileContext(nc) as tc, ExitStack() as ctx:
        dram = ctx.enter_context(tc.tile_pool(name="dram", bufs=1, space="DRAM"))
        x1_d = dram.tile([zc, MUL * 9], f32, kind="ExternalInput", name="x1",
                         uniquify=False)
        x2_d = dram.tile([zc, 9], f32, kind="ExternalInput", name="x2",
                         uniquify=False)
        w_d = dram.tile([zc, MUL * 15], f32, kind="ExternalInput", name="w",
                        uniquify=False)
        out_d = dram.tile([zc, MUL * 9], f32, kind="ExternalOutput", name="out",
                          uniquify=False)

        consts = ctx.enter_context(tc.tile_pool(name="consts", bufs=1))
        ident = consts.tile([128, 128], f32)
        make_identity(nc, ident)
        zeros = consts.tile([128, 128], f32)
        nc.gpsimd.memset(zeros, 0.0)

        io_pool = ctx.enter_context(tc.tile_pool(name="io", bufs=1))
        a_pool = ctx.enter_context(tc.tile_pool(name="apool", bufs=1))
        y_pool = ctx.enter_context(tc.tile_pool(name="ypool", bufs=1))
        o_pool = ctx.enter_context(tc.tile_pool(name="opool", bufs=1))
        diag_pool = ctx.enter_context(tc.tile_pool(name="diagpool", bufs=3))
        ps_pool = ctx.enter_context(tc.tile_pool(name="pspool", bufs=1,
                                                 space="PSUM"))

        x1_v = x1_d.rearrange("(t p s) f -> t p s f", p=128, s=z_in)
        x2_v = x2_d.rearrange("(t p s) f -> t p s f", p=128, s=z_in)
        w_v = w_d.rearrange("(t p s) f -> t p s f", p=128, s=z_in)
        out_v = out_d.rearrange("(t p s) f -> t p s f", p=128, s=z_in)

        a_eng = getattr(nc, a_engine)

        # Y-column ownership: move whole accumulation chains (all terms of one
        # (jg,lo,k) column) to GPSIMD until ~y_gp_frac of Y elements moved.
        col_terms = {}
        for (pi, jg, lo, k, c) in TERMS:
            col_terms.setdefault((jg, lo, k), []).append((pi, c))
        total_terms = len(TERMS)
        gp_cols = set()
        moved = 0
        if y_gp_frac > 0:
            for col, ts_ in sorted(col_terms.items(),
                                   key=lambda kv: -len(kv[1])):
                if moved / total_terms >= y_gp_frac:
                    break
                gp_cols.add(col)
                moved += len(ts_)

        loop_ctx = tc.For_i(0, loop_n, 1) if loop_n > 1 else None
        if loop_ctx is not None:
            loop_ctx.__enter__()
        for t in range(n_super):
            X1 = io_pool.tile([128, z_in, 576], f32, tag="x1")
            W = io_pool.tile([128, z_in, 960], f32, tag="w")
            X2 = io_pool.tile([128, z_in, 9], f32, tag="x2")
            nc.sync.dma_start(out=X1, in_=x1_v[t])
            nc.sync.dma_start(out=W, in_=w_v[t])
            nc.sync.dma_start(out=X2, in_=x2_v[t])

            # ---- A build: A[z, s, (p,i), u] = w[z,s,p,u] * x1[z,s,u,i] ----
            A = a_pool.tile([128, z_in, N_PI, 64], f32, tag="A")
            for p, (l1, l2, lo) in enumerate(INSTR):
                d1 = 2 * l1 + 1
                pi0 = PI_INDEX[(p, 0)]
                out_ap = ap_view(A, pi0 * 64,
                                 [(N_PI * 64, z_in), (64, d1), (1, 64)])
                in0 = ap_view(W, p * 64,
                              [(960, z_in), (0, d1), (1, 64)])
                in1 = ap_view(X1, OFF1[l1],
                              [(576, z_in), (1, d1), (d1, 64)])
                a_eng.tensor_tensor(out=out_ap, in0=in0, in1=in1,
                                    op=AluOp.mult)

            # ---- Y build: Y[z, s, jg, 576@out-layout] += c * A[z, s, pi, u] ----
            # Y[:, s, jg, :] holds sum_{p,i} c * A in the exact output block
            # layout (u-major within each lo block), so PE consumes it as flat
            # contiguous blocks.
            # TensorScalarPtr APs are limited to partition + 2 free dims by the
            # BIR verifier, so each term is one (z_in, 64)-shaped instruction.
            Y = y_pool.tile([128, z_in, 9, 576], f32, tag="Y")
            OFFO = {0: 0, 1: 64, 2: 256}
            written = set()
            # Under k_major, runs with dpi==1 and dk==1 coalesce to 2 free
            # dims on both operands (stride-64 run merges with the contiguous
            # 64-wide column), so they fit the TensorScalarPtr AP limit.
            batched_cols = set()
            if k_major:
                for (jg, lo, c, pi0, k0, dpi, dk, L) in RUNS:
                    if not (dpi == 1 and dk == 1 and L >= 2):
                        continue
                    cols = [(jg, lo, k0 + q) for q in range(L)]
                    st = [col in written for col in cols]
                    seg0 = 0
                    while seg0 < L:
                        s0 = st[seg0]
                        seg1 = seg0 + 1
                        while seg1 < L and st[seg1] == s0:
                            seg1 += 1
                        sl = seg1 - seg0
                        a_ap = ap_view(A, (pi0 + seg0) * 64,
                                       [(N_PI * 64, z_in), (64, sl), (1, 64)])
                        y_ap = ap_view(Y, jg * 576 + OFFO[lo]
                                       + (k0 + seg0) * 64,
                                       [(9 * 576, z_in), (64, sl), (1, 64)])
                        if s0:
                            nc.vector.scalar_tensor_tensor(
                                out=y_ap, in0=a_ap, scalar=c, in1=y_ap,
                                op0=AluOp.mult, op1=AluOp.add)
                        else:
                            nc.vector.tensor_scalar(
                                out=y_ap, in0=a_ap, scalar1=c, scalar2=None,
                                op0=AluOp.mult)
                            written.update(cols[seg0:seg1])
                        seg0 = seg1
                    batched_cols.update(
                        (pi0 + q, jg, lo, k0 + q) for q in range(L))

            n_fw_act = 0
            for (pi, jg, lo, k, c) in TERMS:
                if (pi, jg, lo, k) in batched_cols:
                    continue
                a_ap = ap_view(A, pi * 64, [(N_PI * 64, z_in), (1, 64)])
                if k_major:
                    # contiguous per-(jg,lo,k) column: unit-stride out lets
                    # single-src tensor_scalar first-writes hit the 2x mode
                    y_ap = ap_view(Y, jg * 576 + OFFO[lo] + k * 64,
                                   [(9 * 576, z_in), (1, 64)])
                else:
                    y_ap = ap_view(Y, jg * 576 + OFFO[lo] + k,
                                   [(9 * 576, z_in), (2 * lo + 1, 64)])
                eng = nc.gpsimd if (jg, lo, k) in gp_cols else nc.vector
                if (jg, lo, k) in written:
                    eng.scalar_tensor_tensor(
                        out=y_ap, in0=a_ap, scalar=c, in1=y_ap,
                        op0=AluOp.mult, op1=AluOp.add)
                elif n_fw_act < fw_act_n:
                    # first write of a column as an affine ACT op keeps the
                    # accumulation chains off the (bottleneck) vector engine
                    n_fw_act += 1
                    nc.scalar.activation(
                        out=y_ap, in_=a_ap,
                        func=mybir.ActivationFunctionType.Copy, scale=c)
                    written.add((jg, lo, k))
                else:
                    eng.tensor_scalar(
                        out=y_ap, in0=a_ap, scalar1=c, scalar2=None,
                        op0=AluOp.mult)
                    written.add((jg, lo, k))

            # unreferenced (jg, lo, k) columns would feed garbage into PSUM
            assert len(written) == 9 * 9, (len(written))

            # ---- PE: out_psum[z, s, (u,k)@lo] += diag(x2_j) @ Y_j ----
            # PSUM layout: Ps01 [128, z_in, 256] (lo0|lo1 blocks, 1KB/slot),
            #              Ps2  [128, z_in, 512] (lo2 block padded to a bank)
            Ps01 = ps_pool.tile([128, z_in, 256], f32, tag="ps01")
            Ps2 = ps_pool.tile([128, z_in, 512], f32, tag="ps2")
            OFF01 = {0: 0, 1: 64}

            # start=True resets at bank granularity and PSUM slots share
            # banks, so a per-(s,jg) start would wipe sibling slots — keep
            # the explicit zero pass.
            if True:
                for b in range(z_in * 256 // 512):
                    nc.tensor.matmul(out=ap_view(Ps01, b * 512, [(1, 512)]),
                                     lhsT=zeros, rhs=ap_view(X1, 0, [(1, 512)]),
                                     start=True, stop=False,
                                     skip_group_check=True)
                for s in range(z_in):
                    nc.tensor.matmul(out=ap_view(Ps2, s * 512, [(1, 512)]),
                                     lhsT=zeros, rhs=ap_view(X1, 0, [(1, 512)]),
                                     start=True, stop=False,
                                     skip_group_check=True)

            X2f = X2.rearrange("p s f -> p (s f)")
            n_pe = z_in * 9 * 3
            i_pe = 0
            for s in range(z_in):
                for jg in range(9):
                    dg = diag_pool.tile([128, 128], f32, tag="diag")
                    if diag_engine == "scalar":
                        nc.scalar.activation(
                            out=dg, in_=ident,
                            func=mybir.ActivationFunctionType.Copy,
                            scale=X2f[:, s * 9 + jg:s * 9 + jg + 1])
                    else:
                        getattr(nc, diag_engine).tensor_scalar(
                            out=dg, in0=ident,
                            scalar1=X2f[:, s * 9 + jg:s * 9 + jg + 1],
                            scalar2=None, op0=AluOp.mult)
                    for lo in LS:
                        sz = 64 * (2 * lo + 1)
                        rhs = ap_view(Y, (s * 9 + jg) * 576 + OFFO[lo],
                                      [(1, sz)])
                        if lo < 2:
                            o_ap = ap_view(Ps01, s * 256 + OFF01[lo], [(1, sz)])
                        else:
                            o_ap = ap_view(Ps2, s * 512, [(1, sz)])
                        i_pe += 1
                        # under k_major both Y and PSUM blocks are k-major,
                        # so the flat contiguous views stay correct
                        nc.tensor.matmul(out=o_ap, lhsT=dg, rhs=rhs,
                                         start=False, stop=(i_pe == n_pe),
                                         skip_group_check=True)

            # ---- drain PSUM -> SBUF, store ----
            O = o_pool.tile([128, z_in, 576], f32, tag="O")
            if not k_major:
                nc.scalar.copy(out=ap_view(O, 0, [(576, z_in), (1, 256)]),
                               in_=Ps01)
                nc.scalar.copy(out=ap_view(O, 256, [(576, z_in), (1, 320)]),
                               in_=ap_view(Ps2, 0, [(512, z_in), (1, 320)]))
            else:
                # PSUM blocks are k-major; transpose each back to the u-major
                # output layout during the drain
                for s in range(z_in):
                    nc.scalar.copy(
                        out=ap_view(O, s * 576, [(1, 64)]),
                        in_=ap_view(Ps01, s * 256, [(1, 64)]))
                    for lo in (1, 2):
                        nk = 2 * lo + 1
                        src = (ap_view(Ps01, s * 256 + 64, [(64, nk), (1, 64)])
                               if lo == 1 else
                               ap_view(Ps2, s * 512, [(64, nk), (1, 64)]))
                        nc.scalar.copy(
                            out=ap_view(O, s * 576 + OFFO[lo],
                                        [(1, nk), (nk, 64)]),
                            in_=src)
            nc.sync.dma_start(out=out_v[t], in_=O)

        if loop_ctx is not None:
            loop_ctx.__exit__(None, None, None)

    nc.finalize()
    return nc


# ---- host entry point ----

# A-products on GPSIMD (TensorTensor ucode is legal on Pool; TensorScalarPtr
# is not, so Y stays fully on DVE), diag builds + drains on ACT.  k_major
# stores Y/PSUM blocks k-major so every Y column is contiguous: single-src
# tensor_scalar first-writes hit the DVE 2x port mode (measured ~504us vs
# ~841us per iteration for the u-major layout under identical host load).
BEST_CFG = dict(a_engine="gpsimd", y_gp_frac=0.0, diag_engine="scalar",
                k_major=True)


def kernel(x1: np.ndarray, x2: np.ndarray, w: np.ndarray) -> np.ndarray:
    from concourse.bass_utils import run_bass_kernel_spmd

    x1 = np.ascontiguousarray(x1, dtype=np.float32)
    x2 = np.ascontiguousarray(x2, dtype=np.float32)
    w = np.ascontiguousarray(w, dtype=np.float32)
    assert x1.shape == (Z, 576) and x2.shape == (Z, 9) and w.shape == (Z, 960)

    nc = build_kernel(**BEST_CFG)
    in_maps = []
    for c in range(N_CORES):
        sl = slice(c * ZC, (c + 1) * ZC)
        in_maps.append({"x1": x1[sl], "x2": x2[sl], "w": w[sl]})
    res = run_bass_kernel_spmd(nc, in_maps, core_ids=list(range(N_CORES)))
    return np.concatenate([res.results[c]["out"] for c in range(N_CORES)],
                          axis=0)


# revision 29
# speedup vs baseline: 98.7712x; 1.1175x over previous
"""Trainium2 Bass kernel for nn_CustomWeightedTensorProduct.

Computes, per edge z (Z=32768), an e3nn-style 'uvu' weighted tensor product:
  out[z,u,k@lo] = (1/sqrt(cnt[lo])) * sum_paths w[z,p,u] * C_p[i,j,k] * x1[z,u,i@l1] * x2[z,j@l2]
with MUL=64, l in {0,1,2} (9 spherical components), 15 paths, per-edge weights.

Strategy (per core, data-parallel over z across 8 cores):
  - edges on partitions, z_in=4 edge-slots per partition per super-tile (512 edges)
  - DVE builds A[(p,i),u] = w_p * x1_i products (broadcast APs), then accumulates
    Y_j[(lo,k),u] = sum c * A via scalar_tensor_tensor with immediate CG coeffs
    (batched into runs where (p,i)/(k) advance arithmetically)
  - PE applies the per-edge x2_j factors as diagonal-stationary matmuls,
    accumulating over j in PSUM (zero-pass first, then start=False accumulation)
  - ACT drains PSUM->SBUF; HWDGE DMA moves I/O
"""

import math
import os
import sys
from contextlib import ExitStack

import numpy as np

for _p in ("/opt/trn_rl_repo", "/root/.axon_site/_ro/trn_rl_repo"):
    if os.path.isdir(_p) and _p not in sys.path:
        sys.path.insert(0, _p)

MUL = 64
Z = 32768
N_CORES = 8
ZC = Z // N_CORES          # 4096 edges per core
LS = (0, 1, 2)
INSTR = tuple(sorted((l1, l2, lo) for l1 in LS for l2 in LS for lo in LS
                     if abs(l1 - l2) <= lo <= l1 + l2))
OFF1 = {0: 0, 1: MUL, 2: 4 * MUL}
OFF2 = {0: 0, 1: 1, 2: 4}
CNT = {0: 3, 1: 6, 2: 6}


# ---- real-basis Wigner 3j (identical math to the module's o3.wigner_3j) ----

def _su2_generators(l):
    m = np.arange(-l, l + 1)
    d = 2 * l + 1
    raise_coef = np.sqrt(l * (l + 1) - m[:-1] * (m[:-1] + 1))
    Jp = np.zeros((d, d), complex)
    Jp[np.arange(1, d), np.arange(0, d - 1)] = raise_coef
    Jm = Jp.conj().T
    Jz = np.diag(m).astype(complex)
    return [(Jp + Jm) / 2.0, (Jp - Jm) / 2.0j, Jz]


def _complex_to_real(l):
    d = 2 * l + 1
    U = np.zeros((d, d), complex)
    U[l, l] = 1.0
    s2 = 1.0 / np.sqrt(2.0)
    for m in range(1, l + 1):
        U[l + m, l - m] = s2
        U[l + m, l + m] = (-1) ** m * s2
        U[l - m, l - m] = 1j * s2
        U[l - m, l + m] = -1j * (-1) ** m * s2
    return U


def _real_generators(l):
    U = _complex_to_real(l)
    return [np.real(-1j * (U @ J @ U.conj().T)) for J in _su2_generators(l)]


def _wigner_3j(l1, l2, l3):
    G1, G2, G3 = _real_generators(l1), _real_generators(l2), _real_generators(l3)
    d1, d2, d3 = 2 * l1 + 1, 2 * l2 + 1, 2 * l3 + 1
    I1, I2, I3 = np.eye(d1), np.eye(d2), np.eye(d3)
    rows = []
    for k in range(3):
        rows.append(np.kron(np.kron(G1[k], I2), I3)
                    + np.kron(np.kron(I1, G2[k]), I3)
                    + np.kron(np.kron(I1, I2), G3[k]))
    K = np.concatenate(rows, axis=0)
    _, _, Vh = np.linalg.svd(K)
    C = Vh[-1].reshape(d1, d2, d3)
    C = C / np.linalg.norm(C)
    flat = C.ravel()
    j = int(np.argmax(np.abs(flat)))
    if flat[j] < 0:
        C = -C
    return C


_W3J = {}
for (l1, l2, lo) in INSTR:
    if (l1, l2, lo) not in _W3J:
        _W3J[(l1, l2, lo)] = (_wigner_3j(l1, l2, lo)
                              * math.sqrt(2 * lo + 1)).astype(np.float64)


# ---- compile-time structure: A columns, Y columns, terms, runs ----

def _build_structure():
    # A columns: (p, i) pairs, path-major
    pi_index = {}
    for p, (l1, l2, lo) in enumerate(INSTR):
        for i in range(2 * l1 + 1):
            pi_index[(p, i)] = len(pi_index)
    n_pi = len(pi_index)  # 51

    # terms: (pi, jg, lo, k, c)
    terms = []
    for p, (l1, l2, lo) in enumerate(INSTR):
        C = _W3J[(l1, l2, lo)] / math.sqrt(CNT[lo])
        for i in range(2 * l1 + 1):
            for j in range(2 * l2 + 1):
                for k in range(2 * lo + 1):
                    c = C[i, j, k]
                    if abs(c) > 1e-12:
                        terms.append((pi_index[(p, i)], OFF2[l2] + j, lo, k,
                                      float(c)))

    # run extraction: bucket by (jg, lo, c); greedy arithmetic chains over
    # (pi, k).  (dpi, 0) is excluded: it would hit the same Y column several
    # times within one instruction (overlapping writes).
    buckets = {}
    for (pi, jg, lo, k, c) in terms:
        buckets.setdefault((jg, lo, round(c, 10)), []).append((pi, k, c))
    runs = []  # (jg, lo, c, pi0, k0, dpi, dk, length)
    for (jg, lo, _cr), items in sorted(buckets.items()):
        c = items[0][2]
        left = sorted((pi, k) for (pi, k, _) in items)
        used = set()
        for deltas in ((1, 1), (1, -1), (0, 1)):
            dpi, dk = deltas
            for (pi, k) in left:
                if (pi, k) in used:
                    continue
                # only start a chain at an element with no predecessor
                if (pi - dpi, k - dk) in set(left) - used:
                    continue
                chain = [(pi, k)]
                while True:
                    nxt = (chain[-1][0] + dpi, chain[-1][1] + dk)
                    if nxt in set(left) - used and nxt not in chain:
                        chain.append(nxt)
                    else:
                        break
                if len(chain) >= 2:
                    used.update(chain)
                    runs.append((jg, lo, c, chain[0][0], chain[0][1],
                                 dpi, dk, len(chain)))
        for (pi, k) in left:
            if (pi, k) not in used:
                runs.append((jg, lo, c, pi, k, 0, 0, 1))

    # verify runs reproduce terms exactly
    chk = []
    for (jg, lo, c, pi0, k0, dpi, dk, L) in runs:
        for t in range(L):
            chk.append((pi0 + t * dpi, jg, lo, k0 + t * dk, c))
    assert sorted((a, b, d, e) for (a, b, d, e, _) in chk) == \
           sorted((a, b, d, e) for (a, b, d, e, _) in terms)

    return pi_index, terms, n_pi, runs


PI_INDEX, TERMS, N_PI, RUNS = _build_structure()


# ---- custom DVE op: out = c0*in0 + c1*in1 (fuses two CG terms) ----

_MUL2ADD = None


def _get_mul2add():
    global _MUL2ADD
    if _MUL2ADD is not None:
        return _MUL2ADD
    import re
    from concourse import dve_ops
    from concourse.dve_spec import Spec, Src0, Src1, C0, C1
    from concourse.dve_table_gen import dve_ver_for

    name = "MUL2ADD_ANT"
    if name not in dve_ops._SUB_OPCODE_FOR_NAME:
        dve_ops._SUB_OPCODE_FOR_NAME[name] = 17  # rows 17..0x1f are free
    op = dve_ops.DveOp(
        name,
        Spec(
            body=Src0 * C0 + Src1 * C1,
            reference=lambda in0, in1, s0, s1, imm2:
                in0.astype(np.float32) * s0 + in1.astype(np.float32) * s1,
        ),
        subdim=False,
        uops_sha={},
    )
    ver = dve_ver_for("TRN2")
    try:
        op.compile(ver)
    except ValueError as e:
        m = re.search(r"drifted \(%s: ([0-9a-f]+)" % ver, str(e))
        if not m:
            raise
        op.uops_sha[ver] = m.group(1)
        op.compile(ver)
    if not any(o.name == name for o in dve_ops.OPS):
        dve_ops.OPS.append(op)
    dve_ops.CUSTOM_DVE_SPECS[name] = op.spec
    _MUL2ADD = op
    return op


# ---- the Bass kernel builder ----

def build_kernel(zc=ZC, z_in=4, loop_n=1, a_engine="vector", y_gp_frac=0.0,
                 diag_engine="scalar", fw_act_n=0, k_major=False, fuse2=False):
    import concourse.bass as bass
    import concourse.tile as tile
    from concourse import bacc
    from concourse import mybir
    from concourse.masks import make_identity

    f32 = mybir.dt.float32
    AluOp = mybir.AluOpType
    n_super = zc // (128 * z_in)
    assert zc == n_super * 128 * z_in

    nc = bacc.Bacc("TRN2", target_bir_lowering=False, debug=False)

    def ap_view(t, offset_elems, dims):
        """Manual AP: tile t, extra element offset, free dims [(stride, count)...].
        Partition dim is copied from t."""
        return bass.AP(
            tensor=t.tensor,
            offset=t.offset + offset_elems,
            ap=[list(t.ap[0])] + [[s, n] for (s, n) in dims],
        )

    with tile.TileContext(nc) as tc, ExitStack() as ctx:
        dram = ctx.enter_context(tc.tile_pool(name="dram", bufs=1, space="DRAM"))
        x1_d = dram.tile([zc, MUL * 9], f32, kind="ExternalInput", name="x1",
                         uniquify=False)
        x2_d = dram.tile([zc, 9], f32, kind="ExternalInput", name="x2",
                         uniquify=False)
        w_d = dram.tile([zc, MUL * 15], f32, kind="ExternalInput", name="w",
                        uniquify=False)
        out_d = dram.tile([zc, MUL * 9], f32, kind="ExternalOutput", name="out",
                          uniquify=False)

        consts = ctx.enter_context(tc.tile_pool(name="consts", bufs=1))
        ident = consts.tile([128, 128], f32)
        make_identity(nc, ident)
        zeros = consts.tile([128, 128], f32)
        nc.gpsimd.memset(zeros, 0.0)

        io_pool = ctx.enter_context(tc.tile_pool(name="io", bufs=1))
        a_pool = ctx.enter_context(tc.tile_pool(name="apool", bufs=1))
        y_pool = ctx.enter_context(tc.tile_pool(name="ypool", bufs=1))
        o_pool = ctx.enter_context(tc.tile_pool(name="opool", bufs=1))
        diag_pool = ctx.enter_context(tc.tile_pool(name="diagpool", bufs=3))
        ps_pool = ctx.enter_context(tc.tile_pool(name="pspool", bufs=1,
                                                 space="PSUM"))

        x1_v = x1_d.rearrange("(t p s) f -> t p s f", p=128, s=z_in)
        x2_v = x2_d.rearrange("(t p s) f -> t p s f", p=128, s=z_in)
        w_v = w_d.rearrange("(t p s) f -> t p s f", p=128, s=z_in)
        out_v = out_d.rearrange("(t p s) f -> t p s f", p=128, s=z_in)

        a_eng = getattr(nc, a_engine)

        # Y-column ownership: move whole accumulation chains (all terms of one
        # (jg,lo,k) column) to GPSIMD until ~y_gp_frac of Y elements moved.
        col_terms = {}
        for (pi, jg, lo, k, c) in TERMS:
            col_terms.setdefault((jg, lo, k), []).append((pi, c))
        total_terms = len(TERMS)
        gp_cols = set()
        moved = 0
        if y_gp_frac > 0:
            for col, ts_ in sorted(col_terms.items(),
                                   key=lambda kv: -len(kv[1])):
                if moved / total_terms >= y_gp_frac:
                    break
                gp_cols.add(col)
                moved += len(ts_)

        loop_ctx = tc.For_i(0, loop_n, 1) if loop_n > 1 else None
        if loop_ctx is not None:
            loop_ctx.__enter__()
        for t in range(n_super):
            X1 = io_pool.tile([128, z_in, 576], f32, tag="x1")
            W = io_pool.tile([128, z_in, 960], f32, tag="w")
            X2 = io_pool.tile([128, z_in, 9], f32, tag="x2")
            nc.sync.dma_start(out=X1, in_=x1_v[t])
            nc.sync.dma_start(out=W, in_=w_v[t])
            nc.sync.dma_start(out=X2, in_=x2_v[t])

            # ---- A build: A[z, s, (p,i), u] = w[z,s,p,u] * x1[z,s,u,i] ----
            A = a_pool.tile([128, z_in, N_PI, 64], f32, tag="A")
            for p, (l1, l2, lo) in enumerate(INSTR):
                d1 = 2 * l1 + 1
                pi0 = PI_INDEX[(p, 0)]
                out_ap = ap_view(A, pi0 * 64,
                                 [(N_PI * 64, z_in), (64, d1), (1, 64)])
                in0 = ap_view(W, p * 64,
                              [(960, z_in), (0, d1), (1, 64)])
                in1 = ap_view(X1, OFF1[l1],
                              [(576, z_in), (1, d1), (d1, 64)])
                a_eng.tensor_tensor(out=out_ap, in0=in0, in1=in1,
                                    op=AluOp.mult)

            # ---- Y build: Y[z, s, jg, 576@out-layout] += c * A[z, s, pi, u] ----
            # Y[:, s, jg, :] holds sum_{p,i} c * A in the exact output block
            # layout (u-major within each lo block), so PE consumes it as flat
            # contiguous blocks.
            # TensorScalarPtr APs are limited to partition + 2 free dims by the
            # BIR verifier, so each term is one (z_in, 64)-shaped instruction.
            Y = y_pool.tile([128, z_in, 9, 576], f32, tag="Y")
            OFFO = {0: 0, 1: 64, 2: 256}
            written = set()
            # Under k_major, runs with dpi==1 and dk==1 coalesce to 2 free
            # dims on both operands (stride-64 run merges with the contiguous
            # 64-wide column), so they fit the TensorScalarPtr AP limit.
            batched_cols = set()
            if k_major:
                for (jg, lo, c, pi0, k0, dpi, dk, L) in RUNS:
                    if not (dpi == 1 and dk == 1 and L >= 2):
                        continue
                    cols = [(jg, lo, k0 + q) for q in range(L)]
                    st = [col in written for col in cols]
                    seg0 = 0
                    while seg0 < L:
                        s0 = st[seg0]
                        seg1 = seg0 + 1
                        while seg1 < L and st[seg1] == s0:
                            seg1 += 1
                        sl = seg1 - seg0
                        a_ap = ap_view(A, (pi0 + seg0) * 64,
                                       [(N_PI * 64, z_in), (64, sl), (1, 64)])
                        y_ap = ap_view(Y, jg * 576 + OFFO[lo]
                                       + (k0 + seg0) * 64,
                                       [(9 * 576, z_in), (64, sl), (1, 64)])
                        if s0:
                            nc.vector.scalar_tensor_tensor(
                                out=y_ap, in0=a_ap, scalar=c, in1=y_ap,
                                op0=AluOp.mult, op1=AluOp.add)
                        else:
                            nc.vector.tensor_scalar(
                                out=y_ap, in0=a_ap, scalar1=c, scalar2=None,
                                op0=AluOp.mult)
                            written.update(cols[seg0:seg1])
                        seg0 = seg1
                    batched_cols.update(
                        (pi0 + q, jg, lo, k0 + q) for q in range(L))

            if fuse2:
                assert k_major
                mop = _get_mul2add()
                by_col = {}
                for (pi, jg, lo, k, c) in TERMS:
                    if (pi, jg, lo, k) in batched_cols:
                        continue
                    by_col.setdefault((jg, lo, k), []).append((pi, c))
                for (jg, lo, k), ts_ in sorted(by_col.items()):
                    y_ap = ap_view(Y, jg * 576 + OFFO[lo] + k * 64,
                                   [(9 * 576, z_in), (1, 64)])
                    rest = ts_
                    if (jg, lo, k) not in written:
                        if len(ts_) >= 2:
                            (p1, c1), (p2, c2) = ts_[0], ts_[1]
                            nc.vector._custom_dve(
                                mop, out=y_ap,
                                in0=ap_view(A, p1 * 64,
                                            [(N_PI * 64, z_in), (1, 64)]),
                                in1=ap_view(A, p2 * 64,
                                            [(N_PI * 64, z_in), (1, 64)]),
                                s0=c1, s1=c2)
                            rest = ts_[2:]
                        else:
                            nc.vector.tensor_scalar(
                                out=y_ap,
                                in0=ap_view(A, ts_[0][0] * 64,
                                            [(N_PI * 64, z_in), (1, 64)]),
                                scalar1=ts_[0][1], scalar2=None,
                                op0=AluOp.mult)
                            rest = ts_[1:]
                        written.add((jg, lo, k))
                    for (pi, c) in rest:
                        nc.vector.scalar_tensor_tensor(
                            out=y_ap,
                            in0=ap_view(A, pi * 64,
                                        [(N_PI * 64, z_in), (1, 64)]),
                            scalar=c, in1=y_ap,
                            op0=AluOp.mult, op1=AluOp.add)
                assert len(written) == 81

            n_fw_act = 0
            for (pi, jg, lo, k, c) in TERMS:
                if fuse2 or (pi, jg, lo, k) in batched_cols:
                    continue
                a_ap = ap_view(A, pi * 64, [(N_PI * 64, z_in), (1, 64)])
                if k_major:
                    # contiguous per-(jg,lo,k) column: unit-stride out lets
                    # single-src tensor_scalar first-writes hit the 2x mode
                    y_ap = ap_view(Y, jg * 576 + OFFO[lo] + k * 64,
                                   [(9 * 576, z_in), (1, 64)])
                else:
                    y_ap = ap_view(Y, jg * 576 + OFFO[lo] + k,
                                   [(9 * 576, z_in), (2 * lo + 1, 64)])
                eng = nc.gpsimd if (jg, lo, k) in gp_cols else nc.vector
                if (jg, lo, k) in written:
                    eng.scalar_tensor_tensor(
                        out=y_ap, in0=a_ap, scalar=c, in1=y_ap,
                        op0=AluOp.mult, op1=AluOp.add)
                elif n_fw_act < fw_act_n:
                    # first write of a column as an affine ACT op keeps the
                    # accumulation chains off the (bottleneck) vector engine
                    n_fw_act += 1
                    nc.scalar.activation(
                        out=y_ap, in_=a_ap,
                        func=mybir.ActivationFunctionType.Copy, scale=c)
                    written.add((jg, lo, k))
                else:
                    eng.tensor_scalar(
                        out=y_ap, in0=a_ap, scalar1=c, scalar2=None,
                        op0=AluOp.mult)
                    written.add((jg, lo, k))

            # unreferenced (jg, lo, k) columns would feed garbage into PSUM
            assert len(written) == 9 * 9, (len(written))

            # ---- PE: out_psum[z, s, (u,k)@lo] += diag(x2_j) @ Y_j ----
            # PSUM layout: Ps01 [128, z_in, 256] (lo0|lo1 blocks, 1KB/slot),
            #              Ps2  [128, z_in, 512] (lo2 block padded to a bank)
            Ps01 = ps_pool.tile([128, z_in, 256], f32, tag="ps01")
            Ps2 = ps_pool.tile([128, z_in, 512], f32, tag="ps2")
            OFF01 = {0: 0, 1: 64}

            # start=True resets at bank granularity and PSUM slots share
            # banks, so a per-(s,jg) start would wipe sibling slots — keep
            # the explicit zero pass.
            if True:
                for b in range(z_in * 256 // 512):
                    nc.tensor.matmul(out=ap_view(Ps01, b * 512, [(1, 512)]),
                                     lhsT=zeros, rhs=ap_view(X1, 0, [(1, 512)]),
                                     start=True, stop=False,
                                     skip_group_check=True)
                for s in range(z_in):
                    nc.tensor.matmul(out=ap_view(Ps2, s * 512, [(1, 512)]),
                                     lhsT=zeros, rhs=ap_view(X1, 0, [(1, 512)]),
                                     start=True, stop=False,
                                     skip_group_check=True)

            X2f = X2.rearrange("p s f -> p (s f)")
            n_pe = z_in * 9 * 3
            i_pe = 0
            for s in range(z_in):
                for jg in range(9):
                    dg = diag_pool.tile([128, 128], f32, tag="diag")
                    if diag_engine == "scalar":
                        nc.scalar.activation(
                            out=dg, in_=ident,
                            func=mybir.ActivationFunctionType.Copy,
                            scale=X2f[:, s * 9 + jg:s * 9 + jg + 1])
                    else:
                        getattr(nc, diag_engine).tensor_scalar(
                            out=dg, in0=ident,
                            scalar1=X2f[:, s * 9 + jg:s * 9 + jg + 1],
                            scalar2=None, op0=AluOp.mult)
                    for lo in LS:
                        sz = 64 * (2 * lo + 1)
                        rhs = ap_view(Y, (s * 9 + jg) * 576 + OFFO[lo],
                                      [(1, sz)])
                        if lo < 2:
                            o_ap = ap_view(Ps01, s * 256 + OFF01[lo], [(1, sz)])
                        else:
                            o_ap = ap_view(Ps2, s * 512, [(1, sz)])
                        i_pe += 1
                        # under k_major both Y and PSUM blocks are k-major,
                        # so the flat contiguous views stay correct
                        nc.tensor.matmul(out=o_ap, lhsT=dg, rhs=rhs,
                                         start=False, stop=(i_pe == n_pe),
                                         skip_group_check=True)

            # ---- drain PSUM -> SBUF, store ----
            O = o_pool.tile([128, z_in, 576], f32, tag="O")
            if not k_major:
                nc.scalar.copy(out=ap_view(O, 0, [(576, z_in), (1, 256)]),
                               in_=Ps01)
                nc.scalar.copy(out=ap_view(O, 256, [(576, z_in), (1, 320)]),
                               in_=ap_view(Ps2, 0, [(512, z_in), (1, 320)]))
            else:
                # PSUM blocks are k-major; transpose each back to the u-major
                # output layout during the drain
                for s in range(z_in):
                    nc.scalar.copy(
                        out=ap_view(O, s * 576, [(1, 64)]),
                        in_=ap_view(Ps01, s * 256, [(1, 64)]))
                    for lo in (1, 2):
                        nk = 2 * lo + 1
                        src = (ap_view(Ps01, s * 256 + 64, [(64, nk), (1, 64)])
                               if lo == 1 else
                               ap_view(Ps2, s * 512, [(64, nk), (1, 64)]))
                        nc.scalar.copy(
                            out=ap_view(O, s * 576 + OFFO[lo],
                                        [(1, nk), (nk, 64)]),
                            in_=src)
            nc.sync.dma_start(out=out_v[t], in_=O)

        if loop_ctx is not None:
            loop_ctx.__exit__(None, None, None)

    nc.finalize()
    return nc


# ---- host entry point ----

# A-products on GPSIMD (TensorTensor ucode is legal on Pool; TensorScalarPtr
# is not, so Y stays fully on DVE), diag builds + drains on ACT.  k_major
# stores Y/PSUM blocks k-major so every Y column is contiguous: single-src
# tensor_scalar first-writes hit the DVE 2x port mode (measured ~504us vs
# ~841us per iteration for the u-major layout under identical host load).
BEST_CFG = dict(a_engine="gpsimd", y_gp_frac=0.0, diag_engine="scalar",
                k_major=True)


def kernel(x1: np.ndarray, x2: np.ndarray, w: np.ndarray) -> np.ndarray:
    from concourse.bass_utils import run_bass_kernel_spmd

    x1 = np.ascontiguousarray(x1, dtype=np.float32)
    x2 = np.ascontiguousarray(x2, dtype=np.float32)
    w = np.ascontiguousarray(w, dtype=np.float32)
    assert x1.shape == (Z, 576) and x2.shape == (Z, 9) and w.shape == (Z, 960)

    nc = build_kernel(**BEST_CFG)
    in_maps = []
    for c in range(N_CORES):
        sl = slice(c * ZC, (c + 1) * ZC)
        in_maps.append({"x1": x1[sl], "x2": x2[sl], "w": w[sl]})
    res = run_bass_kernel_spmd(nc, in_maps, core_ids=list(range(N_CORES)))
    return np.concatenate([res.results[c]["out"] for c in range(N_CORES)],
                          axis=0)
